# revision 16
# baseline (speedup 1.0000x reference)
"""Trainium2 Bass kernel for nn_Attention_40510131535961.

The reference module applies softmax over a size-1 axis, so the attention
weights are identically 1.0 and the whole attn MLP (W1/b1/W2/b2, LeakyReLU)
is dead code.  The output reduces to

    context[b, 0, e] = sum_s encode_output[b, s, e]        # [32, 1, 1024]

Strategy: data-parallel over batch across 8 NeuronCores (4 batches/core).
Per core, stream the [4, 2048, 1024] f32 shard through SBUF in 2 MiB DMAs
([128 s-partitions, 4 s-subchunks, 1024 e] tiles), accumulate on VectorE
(fp32 tensor_tensor adds), fold to [128, 1024] per batch, then reduce the
partition axis with a ones-vector matmul on TensorE into PSUM.  The kernel
is HBM-bound: ~32 MiB/core @ ~358 GB/s ≈ 90 us.
"""

import numpy as np

import concourse.bacc as bacc
import concourse.bass as bass
import concourse.mybir as mybir
import concourse.tile as tile
from concourse.bass_utils import run_bass_kernel_spmd

N_CORES = 8
B, S, E = 32, 2048, 1024
BP = B // N_CORES      # batches per core
P = 128                # SBUF partitions
CHUNKS = 4             # DMA chunks per batch
KSUB = S // (CHUNKS * P)  # s-subchunks per chunk (free-dim groups)
F32 = mybir.dt.float32

_CACHE = {}


def _build_nc() -> bass.Bass:
    # Bacc (not raw Bass): its compile()/finalize() runs
    # generate_event_semaphores(), which splits multi-sem waits into
    # InstEventSemaphore — TRN2 instructions support at most 1 wait.
    nc = bacc.Bacc()
    x = nc.declare_dram_parameter("x", [BP, S, E], F32, isOutput=False)
    y = nc.declare_dram_parameter("y", [BP, E], F32, isOutput=True)

    # s = n*P + p  ->  16 s-subchunks of [P, E] per batch
    xr = x[:].rearrange("b (n p) e -> b n p e", p=P)

    # Uniform 2 MiB DMAs keep the HBM stream at ~400 GB/s; each batch tapers
    # its final chunks so the serial tail after the last DMA byte is small.
    # Reduction split: VectorE folds each chunk to width E (in place); the
    # cross-chunk + cross-partition reduction runs on TensorE as ones-matmuls
    # ACCUMULATED in PSUM (start on first chunk, stop on last).  This keeps
    # DVE at ~37 us, gives PE a near-continuous warm stream (~55 us), and the
    # post-last-byte tail is one warm matmul pair + PSUM copies + a 4 KiB DMA.
    PATTERNS = [[4, 4, 4, 2, 1, 1]] * BP
    NH = E // 512  # psum halves per batch

    with tile.TileContext(nc) as tc:
        with (
            tc.tile_pool(name="inp4", bufs=6) as pin4,
            tc.tile_pool(name="inp2", bufs=2) as pin2,
            tc.tile_pool(name="inp1", bufs=4) as pin1,
            tc.tile_pool(name="small", bufs=1) as psm,
            tc.tile_pool(name="ps", bufs=2, space="PSUM") as pps,
        ):
            pool_by_sz = {4: pin4, 2: pin2, 1: pin1}
            ones = psm.tile([P, 1], F32)
            nc.vector.memset(ones[:], 1.0)
            out_sb = psm.tile([1, BP * E], F32)

            for b in range(BP):
                pattern = PATTERNS[b]
                ps_h = [
                    pps.tile([1, 512], F32, tag=f"ps{h}", name=f"ps_{b}_{h}")
                    for h in range(NH)
                ]
                off = 0
                for ci, sz in enumerate(pattern):
                    t = pool_by_sz[sz].tile([P, sz, E], F32, tag=f"c{sz}")
                    nc.sync.dma_start(
                        t[:], xr[b, off : off + sz].rearrange("n p e -> p n e")
                    )
                    off += sz
                    flat = t[:].rearrange("p k e -> p (k e)")
                    # fold chunk to width E in place (sz is a power of two)
                    w = sz * E
                    while w > E:
                        w //= 2
                        nc.vector.tensor_add(
                            flat[:, :w], flat[:, :w], flat[:, w : 2 * w]
                        )
                    for h in range(NH):
                        nc.tensor.matmul(
                            ps_h[h][:],
                            ones[:],
                            flat[:, h * 512 : (h + 1) * 512],
                            start=(ci == 0),
                            stop=(ci == len(pattern) - 1),
                        )
                # copy PSUM out (h0 on ACT, h1 on DVE), DMA the batch row.
                # The output DMA rides the ACT HWDGE ring: SP's queue is
                # FIFO, so nc.sync here would block later input-DMA issues
                # behind this batch's reduction.  (Keep APs 2D: 1D DRAM APs
                # break NEFF load on this stack.)
                for h in range(NH):
                    dst = out_sb[:, b * E + h * 512 : b * E + (h + 1) * 512]
                    if h == 0:
                        nc.scalar.copy(dst, ps_h[h][:])
                    else:
                        nc.vector.tensor_copy(dst, ps_h[h][:])
                nc.scalar.dma_start(y[b : b + 1, :], out_sb[:1, b * E : (b + 1) * E])
    return nc


def _get_nc() -> bass.Bass:
    if "nc" not in _CACHE:
        nc = _build_nc()
        nc.finalize()
        _CACHE["nc"] = nc
    return _CACHE["nc"]


def _run(encode_output: np.ndarray, **spmd_kwargs):
    enc = np.ascontiguousarray(np.asarray(encode_output, dtype=np.float32))
    assert enc.shape == (B, S, E), enc.shape
    in_maps = [{"x": enc[i * BP : (i + 1) * BP]} for i in range(N_CORES)]
    res = run_bass_kernel_spmd(_get_nc(), in_maps, list(range(N_CORES)), **spmd_kwargs)
    out = np.concatenate([res.results[i]["y"] for i in range(N_CORES)], axis=0)
    return out.reshape(B, 1, E), res


def kernel(encode_output, hidden_state=None, W1=None, b1=None, W2=None, b2=None):
    out, _ = _run(encode_output)
    return out
